# revision 74
# baseline (speedup 1.0000x reference)
"""Causal self-attention on 8 Trainium2 NeuronCores.

Sharding: core c handles batch b = c//4 and a group of 4 heads g = c%4
(tensor-parallel over heads x data-parallel over batch). Each core:
  - computes Q/K/V projections for its 256 output dims (4 heads) over its
    batch's 2048 tokens,
  - runs causal attention for its 4 heads: scores in [k, q] layout (no
    transposes), exp on ACT, diagonal-block masking as a 0/1 multiply,
  - PV with the exp'd scores as the STATIONARY operand: y_ps[q, 65] per
    (head, q-block) accumulated over k-blocks; V carries an appended
    ones-column so y_ps[:, 64] is the softmax denominator per-partition,
    making the normalize a cheap per-partition tensor_scalar,
  - PE-transposes the normalized y back to [c, t] for the output
    projection, then applies its 256-row slice of Wo, producing a partial
    fp16 [2048, 1024] output.
Host sums the 4 partials per batch (in fp32) and adds the output bias.

All SBUF operands are fp16 (full PE speed, ~5e-4 element precision); PSUM
accumulation is fp32. Softmax runs without max-subtraction (scores are
bounded by construction: x ~ N(0,1), W ~ 0.02 N(0,1), so |s/sqrt(d)| < ~5).

Schedule: phase 0 computes jt0 Q/K (k-major pairs tracking the x DMA
stream, which arrives in column halves), most of V, and head-0 scores;
phase 1 computes head-1 scores + jt1 Q/K + head-0 PV; heads 2 and 3 run as
one MERGED phase (their score/exp streams interleave per k-block) so the
ACT-bound exp stream overlaps the PE-heavy PV/transpose/output-projection
pipeline, which trails at 1-3 step lags. PSUM: 2 score-slab slots x2 banks,
plus a shared 4-bank pool for PV/transpose/out-projection tiles.

This walrus build only supports ONE sync-wait command per instruction;
_legalize_waits drops transitively-implied waits and moves the rest onto
EventSemaphore carriers, keeping the latest-satisfied wait on the owner so
carriers only ever hold stale (already-fired) waits.
"""

import sys

import numpy as np

try:
    import concourse.bass as bass  # noqa: F401
except ImportError:
    sys.path.insert(0, "/opt/trn_rl_repo")

import concourse.bass as bass
import concourse.mybir as mybir
import concourse.tile as tile
from concourse.bass_utils import run_bass_kernel_spmd

B, T, C, H, D = 2, 2048, 1024, 16, 64
NCORES = 8
HPC = 4          # heads per core
CS = HPC * D     # 256 c-slice per core
KT = C // 128    # 8 contraction tiles for projections
NQ = T // 128    # 16 q/k blocks

F32 = mybir.dt.float32
F16 = mybir.dt.float16
ADD = mybir.AluOpType.add
MULT = mybir.AluOpType.mult
Exp = mybir.ActivationFunctionType.Exp

_PROGRAM = None


def _legalize_waits(nc):
    """This walrus build supports only ONE sync-wait command per compute/DMA
    instruction. Tile's semaphore pass emits waits that are per-proc minimal
    but not transitively minimal, so instructions frequently carry 2-3 waits.

    Pass 1 drops every wait that is transitively implied: we propagate a
    vector-clock "knowledge" set per engine (an engine knows what it waited
    on, plus everything the satisfying updater knew at its update point; an
    engine does NOT implicitly know its own completions, matching the
    issue-runs-ahead hazard model).

    Pass 2 moves any remaining extra waits onto EventSemaphore carrier
    instructions inserted just before the owner on the same engine
    (sequencer-class instructions support standalone waits).
    """
    ok_modes = ("sem-ge-imm",)
    skip_ops = ("EventSemaphore", "Halt")
    cum = {}
    snap = {}      # (sem_id, cum_value) -> knowledge dict {sem_id: value}
    snap_vals = {}  # sem_id -> sorted list of recorded cum values
    upd_pos = {}   # (sem_id, cum_value) -> program position of the update
    K = {}         # proc name -> {sem_id: value}
    es_n = 0
    pos_n = 0
    for f in nc.m.functions:
        for bb in f.blocks:
            new_insts = []
            for inst in bb.instructions:
                si = inst.sync_info
                waits = list(si.on_wait) if si and si.on_wait else []
                updates = list(si.on_update) if si and si.on_update else []
                proc = str(getattr(inst, "engine", "?"))
                kp = K.setdefault(proc, {})
                reducible = (
                    inst.opcode not in skip_ops
                    and all(w.sync_type == "semaphore"
                            and w.wait_mode in ok_modes for w in waits))
                gained = {}
                for w in waits:
                    vals = snap_vals.get(w.id)
                    if not vals:
                        continue
                    import bisect
                    j = bisect.bisect_left(vals, w.wait_value)
                    if j < len(vals):
                        for s, v in snap[(w.id, vals[j])].items():
                            if gained.get(s, -1) < v:
                                gained[s] = v
                    if gained.get(w.id, -1) < w.wait_value:
                        gained[w.id] = w.wait_value
                if reducible and len(waits) > 1:
                    kept = []
                    for w in waits:
                        if kp.get(w.id, -1) >= w.wait_value:
                            continue  # implied by engine knowledge
                        kept.append(w)

                    def sat_pos(w):
                        # program position of the update satisfying w; the
                        # instruction keeps the latest-satisfied wait so the
                        # SEQ-blocking carriers only hold stale waits
                        vals = snap_vals.get(w.id)
                        if not vals:
                            return 1 << 60
                        import bisect
                        j = bisect.bisect_left(vals, w.wait_value)
                        if j >= len(vals):
                            return 1 << 60
                        return upd_pos.get((w.id, vals[j]), 1 << 60)

                    kept.sort(key=sat_pos)
                    # one wait's updater-knowledge may imply another wait
                    changed = True
                    while changed and len(kept) > 1:
                        changed = False
                        for w in list(kept):
                            others = [x for x in kept if x is not w]
                            cover = dict(kp)
                            for x in others:
                                vals = snap_vals.get(x.id)
                                if vals:
                                    import bisect
                                    j = bisect.bisect_left(vals, x.wait_value)
                                    if j < len(vals):
                                        for s, v in snap[(x.id, vals[j])].items():
                                            if cover.get(s, -1) < v:
                                                cover[s] = v
                            if cover.get(w.id, -1) >= w.wait_value:
                                kept.remove(w)
                                changed = True
                                break
                    for w in kept[:-1]:
                        es_n += 1
                        es = mybir.InstEventSemaphore(name=f"eswait-{es_n}")
                        es.engine = inst.engine
                        es.sync_info = type(si)(on_wait=[w], on_update=[])
                        new_insts.append(es)
                    si.on_wait = kept[-1:] if kept else []
                # absorb knowledge (from ALL original waits -- even dropped
                # ones were implied, so this stays monotone and safe)
                for s, v in gained.items():
                    if kp.get(s, -1) < v:
                        kp[s] = v
                for w in waits:
                    if kp.get(w.id, -1) < w.wait_value:
                        kp[w.id] = w.wait_value
                for u in updates:
                    if u.sync_type != "semaphore":
                        continue
                    cum[u.id] = cum.get(u.id, 0) + (u.update_value or 1)
                    s_ = dict(kp)
                    s_[u.id] = cum[u.id]
                    snap[(u.id, cum[u.id])] = s_
                    snap_vals.setdefault(u.id, []).append(cum[u.id])
                    upd_pos[(u.id, cum[u.id])] = pos_n
                new_insts.append(inst)
                pos_n += 1
            bb.instructions[:] = new_insts
    return es_n


def _build_program():
    nc = bass.Bass()
    d = {
        "xT": nc.dram_tensor("xT", [C, T], F16, kind="ExternalInput"),
        "wqT": nc.dram_tensor("wqT", [C, CS], F16, kind="ExternalInput"),
        "wkT": nc.dram_tensor("wkT", [C, CS], F16, kind="ExternalInput"),
        "wvT": nc.dram_tensor("wvT", [C, CS], F16, kind="ExternalInput"),
        "woT": nc.dram_tensor("woT", [CS, C], F16, kind="ExternalInput"),
        "bq2": nc.dram_tensor("bq2", [128, 2], F32, kind="ExternalInput"),
        "bk2": nc.dram_tensor("bk2", [128, 2], F32, kind="ExternalInput"),
        # row 0: spread V bias (65-stride head layout); row 1: all-ones
        "bvo": nc.dram_tensor("bvo", [2, HPC * 65], F32, kind="ExternalInput"),
        "mask": nc.dram_tensor("mask", [128, 128], F16, kind="ExternalInput"),
        "eye": nc.dram_tensor("eye", [128, 128], F16, kind="ExternalInput"),
        "out": nc.dram_tensor("out", [T, C], F16, kind="ExternalOutput"),
    }
    with tile.TileContext(nc) as tc:
        _emit(nc, tc, d)
    n = _legalize_waits(nc)
    if n:
        print(f"kernel: inserted {n} EventSemaphore wait carriers")
    # extended insts (custom DVE ops) need their raw ISA bytes generated;
    # run after the wait edits so the encoding matches final sync_info
    mybir.codegen_inst_isa_subclasses(nc)
    return nc


def _emit(nc, tc, d):
    from contextlib import ExitStack

    es = ExitStack()
    with es:
        p_x = es.enter_context(tc.tile_pool(name="p_x", bufs=1))
        p_qk = es.enter_context(tc.tile_pool(name="p_qk", bufs=1))
        p_v = es.enter_context(tc.tile_pool(name="p_v", bufs=1))
        p_e = es.enter_context(tc.tile_pool(name="p_e", bufs=50))
        p_md = es.enter_context(tc.tile_pool(name="p_md", bufs=30))
        p_y = es.enter_context(tc.tile_pool(name="p_y", bufs=1))
        p_ysb = es.enter_context(tc.tile_pool(name="p_ysb", bufs=1))
        p_r = es.enter_context(tc.tile_pool(name="p_r", bufs=12))
        p_o = es.enter_context(tc.tile_pool(name="p_o", bufs=3))
        p_c = es.enter_context(tc.tile_pool(name="p_c", bufs=1))

        # ---- input loads: weights merged into single DMAs (one HWDGE pass
        # each), ordered so the Q/K path can start as soon as x streams in ----
        def load_w(dname, tag):
            # [KT*128, W] DRAM -> [128, KT*W] SBUF, chunk k at cols W*k
            t_ = p_x.tile([128, KT * CS], F16, tag=tag)
            nc.sync.dma_start(
                out=t_[:].rearrange("p (k c) -> p k c", c=CS),
                in_=d[dname][:].rearrange("(k p) c -> p k c", p=128))
            return t_

        wq_all = load_w("wqT", "wq")
        wk_all = load_w("wkT", "wk")
        # bvo before x: it unblocks the bv broadcast + V-unit DVE chain that
        # would otherwise head-of-line-block the DVE queue mid-startup
        bvo_t = p_c.tile([32, HPC * 65], F32, tag="bvo")
        nc.vector.memset(bvo_t[:], 0.0)
        nc.sync.dma_start(out=bvo_t[0:2, :], in_=d["bvo"][:])
        xt = []

        def load_x(k):
            t_ = p_x.tile([128, T], F16, tag=f"xt{k}")
            nc.sync.dma_start(out=t_[:],
                              in_=d["xT"][128 * k:128 * (k + 1), :])
            xt.append(t_)

        for k in range(KT):
            load_x(k)
        bq_t = p_c.tile([128, 2], F32, tag="bq")
        nc.sync.dma_start(out=bq_t[:], in_=d["bq2"][:])
        bk_t = p_c.tile([128, 2], F32, tag="bk")
        nc.sync.dma_start(out=bk_t[:], in_=d["bk2"][:])
        mask_t = p_c.tile([128, 128], F16, tag="mask")
        nc.sync.dma_start(out=mask_t[:], in_=d["mask"][:])
        eye_t = p_c.tile([128, 128], F16, tag="eye")
        nc.sync.dma_start(out=eye_t[:], in_=d["eye"][:])
        wv_all = load_w("wvT", "wv")
        wo_all = p_x.tile([128, 2 * C], F16, tag="wo")
        nc.sync.dma_start(
            out=wo_all[:].rearrange("p (ct c) -> p ct c", c=C),
            in_=d["woT"][:].rearrange("(ct p) c -> p ct c", p=128))


        # ---- engine warm-up ----
        # rows for PE row-broadcast matmuls: row 0 = 1.0, rows 1-31 = 0.0
        # (K=1 matmuls round the contraction up to 32 partitions, so the
        # stationary operand must be explicitly zero on the padding rows)
        ones_t = p_c.tile([32, 128], F32, tag="ones")
        nc.vector.memset(ones_t[:], 0.0)
        nc.vector.memset(ones_t[0:1, :], 1.0)
        # ACT: absorb a DVE tick, then the activation-const DMA tick
        aw1 = p_c.tile([128, 2], F32, tag="aw1")
        nc.vector.memset(aw1[:], 0.0)
        aw2 = p_c.tile([128, 2], F32, tag="aw2")
        nc.scalar.copy(out=aw2[:], in_=aw1[:])
        aw3 = p_c.tile([128, 2], F32, tag="aw3")
        nc.scalar.activation(out=aw3[:], in_=aw2[:], func=Exp)

        qT = [p_qk.tile([128, T], F16, tag=f"qT{jt}", name=f"qT{jt}")
              for jt in range(2)]
        kTt = [p_qk.tile([128, T], F16, tag=f"kT{jt}", name=f"kT{jt}")
               for jt in range(2)]
        yT = [p_y.tile([128, T], F16, tag=f"yT{ct}", name=f"yT{ct}")
              for ct in range(2)]
        bv_t = p_c.tile([128, HPC * 65], F32, tag="bv")
        v_sb = [None] * NQ
        y_sb = {}      # (j, ct) -> [128, 128] f16 normalized y (q, cs-half)

        E_chunks = {}  # (h, i) -> list of (c0, cw, tile)
        E_masks = {}   # (h, i) -> masked diagonal [128, 128]

        def qk_move(ps, jt, tt, which):
            b_tile = bq_t if which == "q" else bk_t
            dest = qT if which == "q" else kTt
            nc.vector.tensor_scalar(
                out=dest[jt][:, 512 * tt:512 * (tt + 1)], in0=ps[:],
                scalar1=b_tile[:, jt:jt + 1], scalar2=None, op0=ADD)

        def emit_QK_group(ps_qk, jt, tt, which, tag="qk"):
            w_all = wq_all if which == "q" else wk_all
            ps = ps_qk.tile([128, 512], F32, tag=tag, name="psqk")
            for k in range(KT):
                nc.tensor.matmul(
                    out=ps[:],
                    lhsT=w_all[:, CS * k + 128 * jt:CS * k + 128 * (jt + 1)],
                    rhs=xt[k][:, 512 * tt:512 * (tt + 1)],
                    start=(k == 0), stop=(k == KT - 1),
                    skip_group_check=True)
            qk_move(ps, jt, tt, which)

        def emit_QK_pair_kmajor(ps_qk, jt, tt):
            """q and k groups for (jt, tt) with their k-loops interleaved so
            both track the xt[k] DMA arrivals (startup path)."""
            psq = ps_qk.tile([128, 512], F32, tag="qk", name="psq")
            psk = ps_qk.tile([128, 512], F32, tag="qk", name="psk")
            for k in range(KT):
                for ps, w_all in ((psq, wq_all), (psk, wk_all)):
                    nc.tensor.matmul(
                        out=ps[:],
                        lhsT=w_all[:, CS * k + 128 * jt:
                                   CS * k + 128 * (jt + 1)],
                        rhs=xt[k][:, 512 * tt:512 * (tt + 1)],
                        start=(k == 0), stop=(k == KT - 1),
                        skip_group_check=True)
            qk_move(psq, jt, tt, "q")
            qk_move(psk, jt, tt, "k")

        def emit_V_unit(ps_v, tt, tag="v"):
            ps = ps_v.tile([128, CS], F32, tag=tag, name="psv")
            for k in range(KT):
                nc.tensor.matmul(
                    out=ps[:],
                    lhsT=xt[k][:, 128 * tt:128 * (tt + 1)],
                    rhs=wv_all[:, CS * k:CS * (k + 1)],
                    start=(k == 0), stop=(k == KT - 1),
                    skip_group_check=True)
            vt = p_v.tile([128, HPC * 65], F16, tag=f"vt{tt}",
                          name=f"vt{tt}")
            nc.vector.tensor_tensor(
                out=vt[:].rearrange("p (h d) -> p h d", d=65)[:, :, 0:64],
                in0=ps[:].rearrange("p (h d) -> p h d", d=64),
                in1=bv_t[:].rearrange("p (h d) -> p h d", d=65)[:, :, 0:64],
                op=ADD)
            nc.vector.memset(
                vt[:].rearrange("p (h d) -> p h d", d=65)[:, :, 64:65], 1.0)
            v_sb[tt] = vt

        def emit_ST(ps_st, h, i, c0, c1):
            """Scores+exp for head h, k-row-block i, score columns [c0, c1).
            c0 must be max(qlo, c0)-normalized by the caller."""
            jt, hb = h // 2, 64 * (h % 2)
            qlo = 128 * i
            c0 = max(qlo, c0)
            cw = c1 - c0
            if cw <= 0:
                return
            slab = ps_st.tile([128, cw], F32, tag="st", name="slab")
            n0 = c0
            while n0 < c1:
                nw = min(512, c1 - n0)
                nc.tensor.matmul(
                    out=slab[:, n0 - c0:n0 - c0 + nw],
                    lhsT=kTt[jt][hb:hb + 64, qlo:qlo + 128],
                    rhs=qT[jt][hb:hb + 64, n0:n0 + nw],
                    start=True, stop=True, skip_group_check=True)
                n0 += nw
            e = p_e.tile([128, cw], F16, tag="E", name="e")
            nc.scalar.activation(out=e[:], in_=slab[:], func=Exp)
            if c0 == qlo:  # diagonal chunk: masked copy into its own tile
                md = p_md.tile([128, 128], F16, tag="md", name="md")
                nc.vector.tensor_tensor(out=md[:], in0=e[:, 0:128],
                                        in1=mask_t[:], op=MULT)
                E_masks[(h, i)] = md
            E_chunks.setdefault((h, i), []).append((c0, cw, e))

        def emit_PV_j(ps_pv, h, j):
            """y_ps[q=128, 65] for head h, q-block j, accumulated over
            k-blocks i=0..j with the exp'd scores as the stationary operand.
            Column 64 (V's ones-column) accumulates the softmax denominator."""
            qlo = 128 * j
            pvt = ps_pv.tile([128, 512], F32, tag="pv", name="pv")
            for i in range(j + 1):
                if i == j:
                    lhs = E_masks[(h, i)][:]
                else:
                    c0, cw, e = next(ch for ch in E_chunks[(h, i)]
                                     if ch[0] <= qlo < ch[0] + ch[1])
                    lhs = e[:, qlo - c0:qlo - c0 + 128]
                nc.tensor.matmul(
                    out=pvt[:, 0:65], lhsT=lhs,
                    rhs=v_sb[i][:, 65 * h:65 * h + 65],
                    start=(i == 0), stop=(i == j), skip_group_check=True)
            # normalize: per-partition (q) reciprocal of the denominator col
            rcp = p_r.tile([128, 1], F32, tag="rcp", name="rcp")
            nc.vector.reciprocal_approx_fast(out=rcp[:], in_=pvt[:, 64:65])
            ct, half = h // 2, 64 * (h % 2)
            key = (j, ct)
            if key not in y_sb:
                y_sb[key] = p_ysb.tile([128, 128], F16, tag=f"ysb{j}_{ct}",
                                       name=f"ysb{j}_{ct}")
            nc.vector.tensor_scalar(
                out=y_sb[key][:, half:half + 64], in0=pvt[:, 0:64],
                scalar1=rcp[:], scalar2=None, op0=MULT)

        def emit_transpose(ps_tr, j, ct):
            """yT[ct][:, 128j:128j+128] = transpose of y_sb[(j, ct)]."""
            tp = ps_tr.tile([128, 128], F16, tag="pv", name="tr")
            nc.tensor.transpose(out=tp[:], in_=y_sb[(j, ct)][:],
                                identity=eye_t[:])
            nc.vector.tensor_copy(out=yT[ct][:, 128 * j:128 * (j + 1)],
                                  in_=tp[:])

        def emit_out(ps_pv, tt, direct=False):
            # two half-tiles from the pv pool keep the slab pool's rotation
            # free of out-projection traffic; the two PSUM->SBUF halves run
            # concurrently on ACT and DVE
            pos = [ps_pv.tile([128, 512], F32, tag="pv", name=f"po{jt}")
                   for jt in range(2)]
            for jt in range(2):
                for ct in range(2):
                    nc.tensor.matmul(
                        out=pos[jt][:],
                        lhsT=yT[ct][:, 128 * tt:128 * (tt + 1)],
                        rhs=wo_all[:, C * ct + 512 * jt:
                                   C * ct + 512 * (jt + 1)],
                        start=(ct == 0), stop=(ct == 1),
                        skip_group_check=True)
            stg = p_o.tile([128, 1024], F16, tag="o", name="stg")
            nc.scalar.copy(out=stg[:, 0:512], in_=pos[0][:])
            nc.vector.tensor_copy(out=stg[:, 512:1024], in_=pos[1][:])
            nc.sync.dma_start(out=d["out"][128 * tt:128 * (tt + 1), :],
                              in_=stg[:])

        es_st = ExitStack()
        es_v = ExitStack()
        es_pv = ExitStack()
        with es_st, es_pv:
            ps_st = es_st.enter_context(
                tc.tile_pool(name="ps_st", bufs=2, space="PSUM"))
            with es_v:
                ps_qk = es_v.enter_context(
                    tc.tile_pool(name="ps_qk", bufs=2, space="PSUM"))
                ps_v = es_v.enter_context(
                    tc.tile_pool(name="ps_v", bufs=2, space="PSUM"))
                # absorb the bvo DMA tick on PE, then broadcast V-bias row
                # across 128 partitions with a K=1 ones matmul
                bv_ps = ps_v.tile([128, HPC * 65], F32, tag="v",
                                  name="bvps")
                nc.tensor.matmul(out=bv_ps[:], lhsT=ones_t[:],
                                 rhs=bvo_t[:], start=True, stop=True,
                                 skip_group_check=True)
                nc.vector.tensor_copy(out=bv_t[:], in_=bv_ps[:])
                # Q/K for jt0 tt0/tt1 first (unblocks head-0 scores panel 0);
                # the first pair runs k-major so it tracks the xt DMA stream
                emit_QK_pair_kmajor(ps_qk, 0, 0)
                # head-0 scores interleaved with the remaining jt0 QK groups
                # and the V units (jt1 QK moves to phase 1 where ACT is the
                # pacer and PE has slack)
                jt1_pool = [None]
                fillers = []
                for tt in (2, 3):
                    fillers.append(lambda tt=tt: emit_QK_group(
                        ps_qk, 0, tt, "q"))
                    fillers.append(lambda tt=tt: emit_QK_group(
                        ps_qk, 0, tt, "k"))
                for tt in range(12):
                    fillers.append(lambda tt=tt: emit_V_unit(ps_v, tt))
                jt1 = []
                for tt in range(4):
                    jt1.append(lambda tt=tt: emit_QK_group(
                        jt1_pool[0], 1, tt, "q", tag="pv"))
                    jt1.append(lambda tt=tt: emit_QK_group(
                        jt1_pool[0], 1, tt, "k", tag="pv"))
                fi = iter(fillers)

                def fill(n):
                    for _ in range(n):
                        f = next(fi, None)
                        if f:
                            f()

                # starter panels for i<4 need only the tt0 moves, so ACT
                # begins exp'ing right after the first QK pair lands
                for i in range(4):
                    emit_ST(ps_st, 0, i, 0, 512)
                emit_QK_pair_kmajor(ps_qk, 0, 1)
                for i in range(4):
                    emit_ST(ps_st, 0, i, 512, 1024)
                    fill(1)
                for i in range(4, 8):
                    emit_ST(ps_st, 0, i, 0, 1024)
                    fill(2)
                for i in range(NQ):
                    emit_ST(ps_st, 0, i, 1024, 2048)
                    fill(1)
                fill(len(fillers))
            # ps_v closed; phase 1 (head 1 scores + head-0 PV + jt1 QK)
            ps_pv = es_pv.enter_context(
                tc.tile_pool(name="ps_pv", bufs=4, space="PSUM"))
            jt1_pool[0] = ps_pv
            fj = iter(jt1)
            for i in range(NQ):
                emit_ST(ps_st, 1, i, 0, 1024)
                emit_ST(ps_st, 1, i, 1024, 2048)
                if i < 4:
                    emit_V_unit(ps_pv, 12 + i, tag="pv")
                f = next(fj, None)
                if f:
                    f()
                if i < 8:
                    for j in (2 * i, 2 * i + 1):
                        emit_PV_j(ps_pv, 0, j)
            for f in fj:
                f()
            ps_tr = ps_pv
            # merged phase 2+3: head-2 and head-3 scores interleave per
            # k-block, so the ACT-bound exp stream of both heads overlaps the
            # PE-heavy PV/transpose/out pipeline instead of serializing it
            tr0 = 0  # next (j, ct=0) transpose to emit
            for i in range(NQ):
                if i >= 3:
                    # deps are 2+ steps old: placing it first gives PE ready
                    # work while the new slabs wait on exp slot recycling
                    emit_out(ps_pv, i - 3)
                if i >= 1:
                    j = i - 1
                    emit_PV_j(ps_pv, 2, j)
                    emit_PV_j(ps_pv, 3, j)
                emit_ST(ps_st, 2, i, 0, 1024)
                emit_ST(ps_st, 2, i, 1024, 2048)
                emit_ST(ps_st, 3, i, 0, 1024)
                emit_ST(ps_st, 3, i, 1024, 2048)
                if i < 8:
                    for j in (2 * i, 2 * i + 1):
                        emit_PV_j(ps_pv, 1, j)
                if i >= 2:
                    emit_transpose(ps_tr, i - 2, 1)
                # transpose (j, 0) once heads 0 and 1 are normed
                while tr0 < NQ and tr0 <= 2 * i - 1:
                    emit_transpose(ps_tr, tr0, 0)
                    tr0 += 1
            emit_PV_j(ps_pv, 2, 15)
            emit_PV_j(ps_pv, 3, 15)
            for j in (14, 15):
                emit_transpose(ps_tr, j, 1)
            for tt in (13, 14, 15):
                emit_out(ps_pv, tt)


def _core_inputs(x, Wq, bq, Wk, bk, Wv, bv, Wo, core):
    b, g = core // HPC, core % HPC
    hs = slice(CS * g, CS * (g + 1))
    scale = np.float32(1.0 / np.sqrt(D))
    bvo = np.zeros((2, HPC * 65), np.float32)
    bvo[1, :] = 1.0
    for h in range(HPC):
        bvo[0, 65 * h:65 * h + 64] = bv[hs][64 * h:64 * h + 64]
    kk = np.arange(128)[:, None]
    qq = np.arange(128)[None, :]
    mask01 = (kk <= qq).astype(np.float16)
    return {
        "xT": np.ascontiguousarray(x[b].T).astype(np.float16),
        "wqT": np.ascontiguousarray((Wq[hs] * scale).T).astype(np.float16),
        "wkT": np.ascontiguousarray(Wk[hs].T).astype(np.float16),
        "wvT": np.ascontiguousarray(Wv[hs].T).astype(np.float16),
        "woT": np.ascontiguousarray(Wo[:, hs].T).astype(np.float16),
        "bq2": np.ascontiguousarray((bq[hs] * scale).reshape(2, 128).T,
                                    np.float32),
        "bk2": np.ascontiguousarray(bk[hs].reshape(2, 128).T, np.float32),
        "bvo": bvo,
        "mask": mask01,
        "eye": np.eye(128, dtype=np.float16),
    }


def get_program():
    global _PROGRAM
    if _PROGRAM is None:
        _PROGRAM = _build_program()
    return _PROGRAM


def make_in_maps(x, Wq, bq, Wk, bk, Wv, bv, Wo):
    return [_core_inputs(x, Wq, bq, Wk, bk, Wv, bv, Wo, core)
            for core in range(NCORES)]


def assemble(results, bo):
    out = np.zeros((B, T, C), np.float32)
    for core in range(NCORES):
        out[core // HPC] += results[core]["out"].astype(np.float32)
    out += bo[None, None, :]
    return out


def kernel(x, Wq, bq, Wk, bk, Wv, bv, Wo, bo):
    x = np.asarray(x, np.float32)
    Wq, bq = np.asarray(Wq, np.float32), np.asarray(bq, np.float32)
    Wk, bk = np.asarray(Wk, np.float32), np.asarray(bk, np.float32)
    Wv, bv = np.asarray(Wv, np.float32), np.asarray(bv, np.float32)
    Wo, bo = np.asarray(Wo, np.float32), np.asarray(bo, np.float32)
    nc = get_program()
    in_maps = make_in_maps(x, Wq, bq, Wk, bk, Wv, bv, Wo)
    res = run_bass_kernel_spmd(nc, in_maps, list(range(NCORES)))
    return assemble(res.results, bo)


# revision 81
# speedup vs baseline: 1.0021x; 1.0021x over previous
"""Causal self-attention on 8 Trainium2 NeuronCores.

Sharding: core c handles batch b = c//4 and a group of 4 heads g = c%4
(tensor-parallel over heads x data-parallel over batch). Each core:
  - computes Q/K/V projections for its 256 output dims (4 heads) over its
    batch's 2048 tokens,
  - runs causal attention for its 4 heads: scores in [k, q] layout (no
    transposes), exp on ACT, diagonal-block masking as a 0/1 multiply,
  - PV with the exp'd scores as the STATIONARY operand: y_ps[q, 65] per
    (head, q-block) accumulated over k-blocks; V carries an appended
    ones-column so y_ps[:, 64] is the softmax denominator per-partition,
    making the normalize a cheap per-partition tensor_scalar,
  - PE-transposes the normalized y back to [c, t] for the output
    projection, then applies its 256-row slice of Wo, producing a partial
    fp16 [2048, 1024] output.
Host sums the 4 partials per batch (in fp32) and adds the output bias.

All SBUF operands are fp16 (full PE speed, ~5e-4 element precision); PSUM
accumulation is fp32. Softmax runs without max-subtraction (scores are
bounded by construction: x ~ N(0,1), W ~ 0.02 N(0,1), so |s/sqrt(d)| < ~5).

Schedule: phase 0 computes jt0 Q/K (k-major pairs tracking the x DMA
stream, which arrives in column halves), most of V, and head-0 scores;
phase 1 computes head-1 scores + jt1 Q/K + head-0 PV; heads 2 and 3 run as
one MERGED phase (their score/exp streams interleave per k-block) so the
ACT-bound exp stream overlaps the PE-heavy PV/transpose/output-projection
pipeline, which trails at 1-3 step lags. PSUM: 2 score-slab slots x2 banks,
plus a shared 4-bank pool for PV/transpose/out-projection tiles.

This walrus build only supports ONE sync-wait command per instruction;
_legalize_waits drops transitively-implied waits and moves the rest onto
EventSemaphore carriers, keeping the latest-satisfied wait on the owner so
carriers only ever hold stale (already-fired) waits.
"""

import sys

import numpy as np

try:
    import concourse.bass as bass  # noqa: F401
except ImportError:
    sys.path.insert(0, "/opt/trn_rl_repo")

import concourse.bass as bass
import concourse.mybir as mybir
import concourse.tile as tile
from concourse.bass_utils import run_bass_kernel_spmd

B, T, C, H, D = 2, 2048, 1024, 16, 64
NCORES = 8
HPC = 4          # heads per core
CS = HPC * D     # 256 c-slice per core
KT = C // 128    # 8 contraction tiles for projections
NQ = T // 128    # 16 q/k blocks

F32 = mybir.dt.float32
F16 = mybir.dt.float16
ADD = mybir.AluOpType.add
MULT = mybir.AluOpType.mult
Exp = mybir.ActivationFunctionType.Exp

_PROGRAM = None


def _legalize_waits(nc):
    """This walrus build supports only ONE sync-wait command per compute/DMA
    instruction. Tile's semaphore pass emits waits that are per-proc minimal
    but not transitively minimal, so instructions frequently carry 2-3 waits.

    Pass 1 drops every wait that is transitively implied: we propagate a
    vector-clock "knowledge" set per engine (an engine knows what it waited
    on, plus everything the satisfying updater knew at its update point; an
    engine does NOT implicitly know its own completions, matching the
    issue-runs-ahead hazard model).

    Pass 2 moves any remaining extra waits onto EventSemaphore carrier
    instructions inserted just before the owner on the same engine
    (sequencer-class instructions support standalone waits).
    """
    ok_modes = ("sem-ge-imm",)
    skip_ops = ("EventSemaphore", "Halt")
    cum = {}
    snap = {}      # (sem_id, cum_value) -> knowledge dict {sem_id: value}
    snap_vals = {}  # sem_id -> sorted list of recorded cum values
    upd_pos = {}   # (sem_id, cum_value) -> program position of the update
    K = {}         # proc name -> {sem_id: value}
    es_n = 0
    pos_n = 0
    for f in nc.m.functions:
        for bb in f.blocks:
            new_insts = []
            for inst in bb.instructions:
                si = inst.sync_info
                waits = list(si.on_wait) if si and si.on_wait else []
                updates = list(si.on_update) if si and si.on_update else []
                proc = str(getattr(inst, "engine", "?"))
                kp = K.setdefault(proc, {})
                reducible = (
                    inst.opcode not in skip_ops
                    and all(w.sync_type == "semaphore"
                            and w.wait_mode in ok_modes for w in waits))
                gained = {}
                for w in waits:
                    vals = snap_vals.get(w.id)
                    if not vals:
                        continue
                    import bisect
                    j = bisect.bisect_left(vals, w.wait_value)
                    if j < len(vals):
                        for s, v in snap[(w.id, vals[j])].items():
                            if gained.get(s, -1) < v:
                                gained[s] = v
                    if gained.get(w.id, -1) < w.wait_value:
                        gained[w.id] = w.wait_value
                if reducible and len(waits) > 1:
                    kept = []
                    for w in waits:
                        if kp.get(w.id, -1) >= w.wait_value:
                            continue  # implied by engine knowledge
                        kept.append(w)

                    def sat_pos(w):
                        # program position of the update satisfying w; the
                        # instruction keeps the latest-satisfied wait so the
                        # SEQ-blocking carriers only hold stale waits
                        vals = snap_vals.get(w.id)
                        if not vals:
                            return 1 << 60
                        import bisect
                        j = bisect.bisect_left(vals, w.wait_value)
                        if j >= len(vals):
                            return 1 << 60
                        return upd_pos.get((w.id, vals[j]), 1 << 60)

                    kept.sort(key=sat_pos)
                    # one wait's updater-knowledge may imply another wait
                    changed = True
                    while changed and len(kept) > 1:
                        changed = False
                        for w in list(kept):
                            others = [x for x in kept if x is not w]
                            cover = dict(kp)
                            for x in others:
                                vals = snap_vals.get(x.id)
                                if vals:
                                    import bisect
                                    j = bisect.bisect_left(vals, x.wait_value)
                                    if j < len(vals):
                                        for s, v in snap[(x.id, vals[j])].items():
                                            if cover.get(s, -1) < v:
                                                cover[s] = v
                            if cover.get(w.id, -1) >= w.wait_value:
                                kept.remove(w)
                                changed = True
                                break
                    for w in kept[:-1]:
                        es_n += 1
                        es = mybir.InstEventSemaphore(name=f"eswait-{es_n}")
                        es.engine = inst.engine
                        es.sync_info = type(si)(on_wait=[w], on_update=[])
                        new_insts.append(es)
                    si.on_wait = kept[-1:] if kept else []
                # absorb knowledge (from ALL original waits -- even dropped
                # ones were implied, so this stays monotone and safe)
                for s, v in gained.items():
                    if kp.get(s, -1) < v:
                        kp[s] = v
                for w in waits:
                    if kp.get(w.id, -1) < w.wait_value:
                        kp[w.id] = w.wait_value
                for u in updates:
                    if u.sync_type != "semaphore":
                        continue
                    cum[u.id] = cum.get(u.id, 0) + (u.update_value or 1)
                    s_ = dict(kp)
                    s_[u.id] = cum[u.id]
                    snap[(u.id, cum[u.id])] = s_
                    snap_vals.setdefault(u.id, []).append(cum[u.id])
                    upd_pos[(u.id, cum[u.id])] = pos_n
                new_insts.append(inst)
                pos_n += 1
            bb.instructions[:] = new_insts
    return es_n


def _build_program():
    nc = bass.Bass()
    d = {
        "xT": nc.dram_tensor("xT", [C, T], F16, kind="ExternalInput"),
        "wqT": nc.dram_tensor("wqT", [C, CS], F16, kind="ExternalInput"),
        "wkT": nc.dram_tensor("wkT", [C, CS], F16, kind="ExternalInput"),
        "wvT": nc.dram_tensor("wvT", [C, CS], F16, kind="ExternalInput"),
        "woT": nc.dram_tensor("woT", [CS, C], F16, kind="ExternalInput"),
        "bq2": nc.dram_tensor("bq2", [128, 2], F32, kind="ExternalInput"),
        "bk2": nc.dram_tensor("bk2", [128, 2], F32, kind="ExternalInput"),
        # row 0: spread V bias (65-stride head layout); row 1: all-ones
        "bvo": nc.dram_tensor("bvo", [2, HPC * 65], F32, kind="ExternalInput"),
        "mask": nc.dram_tensor("mask", [128, 128], F16, kind="ExternalInput"),
        "eye": nc.dram_tensor("eye", [128, 128], F16, kind="ExternalInput"),
        "out": nc.dram_tensor("out", [T, C], F16, kind="ExternalOutput"),
    }
    with tile.TileContext(nc) as tc:
        _emit(nc, tc, d)
    n = _legalize_waits(nc)
    if n:
        print(f"kernel: inserted {n} EventSemaphore wait carriers")
    # extended insts (custom DVE ops) need their raw ISA bytes generated;
    # run after the wait edits so the encoding matches final sync_info
    mybir.codegen_inst_isa_subclasses(nc)
    return nc


def _emit(nc, tc, d):
    from contextlib import ExitStack

    es = ExitStack()
    with es:
        p_x = es.enter_context(tc.tile_pool(name="p_x", bufs=1))
        p_qk = es.enter_context(tc.tile_pool(name="p_qk", bufs=1))
        p_v = es.enter_context(tc.tile_pool(name="p_v", bufs=1))
        p_e = es.enter_context(tc.tile_pool(name="p_e", bufs=50))
        p_md = es.enter_context(tc.tile_pool(name="p_md", bufs=30))
        p_y = es.enter_context(tc.tile_pool(name="p_y", bufs=1))
        p_ysb = es.enter_context(tc.tile_pool(name="p_ysb", bufs=1))
        p_r = es.enter_context(tc.tile_pool(name="p_r", bufs=12))
        p_o = es.enter_context(tc.tile_pool(name="p_o", bufs=3))
        p_c = es.enter_context(tc.tile_pool(name="p_c", bufs=1))

        # ---- input loads: weights merged into single DMAs (one HWDGE pass
        # each), ordered so the Q/K path can start as soon as x streams in ----
        def load_w(dname, tag):
            # [KT*128, W] DRAM -> [128, KT*W] SBUF, chunk k at cols W*k
            t_ = p_x.tile([128, KT * CS], F16, tag=tag)
            nc.sync.dma_start(
                out=t_[:].rearrange("p (k c) -> p k c", c=CS),
                in_=d[dname][:].rearrange("(k p) c -> p k c", p=128))
            return t_

        wq_all = load_w("wqT", "wq")
        wk_all = load_w("wkT", "wk")
        # bvo before x: it unblocks the bv broadcast + V-unit DVE chain that
        # would otherwise head-of-line-block the DVE queue mid-startup
        bvo_t = p_c.tile([32, HPC * 65], F32, tag="bvo")
        nc.vector.memset(bvo_t[:], 0.0)
        nc.sync.dma_start(out=bvo_t[0:2, :], in_=d["bvo"][:])
        xt = []

        def load_x(k):
            t_ = p_x.tile([128, T], F16, tag=f"xt{k}")
            nc.sync.dma_start(out=t_[:],
                              in_=d["xT"][128 * k:128 * (k + 1), :])
            xt.append(t_)

        for k in range(KT):
            load_x(k)
        bq_t = p_c.tile([128, 2], F32, tag="bq")
        nc.sync.dma_start(out=bq_t[:], in_=d["bq2"][:])
        bk_t = p_c.tile([128, 2], F32, tag="bk")
        nc.sync.dma_start(out=bk_t[:], in_=d["bk2"][:])
        mask_t = p_c.tile([128, 128], F16, tag="mask")
        nc.sync.dma_start(out=mask_t[:], in_=d["mask"][:])
        eye_t = p_c.tile([128, 128], F16, tag="eye")
        nc.sync.dma_start(out=eye_t[:], in_=d["eye"][:])
        wv_all = load_w("wvT", "wv")
        wo_all = p_x.tile([128, 2 * C], F16, tag="wo")
        nc.sync.dma_start(
            out=wo_all[:].rearrange("p (ct c) -> p ct c", c=C),
            in_=d["woT"][:].rearrange("(ct p) c -> p ct c", p=128))


        # ---- engine warm-up ----
        # rows for PE row-broadcast matmuls: row 0 = 1.0, rows 1-31 = 0.0
        # (K=1 matmuls round the contraction up to 32 partitions, so the
        # stationary operand must be explicitly zero on the padding rows)
        ones_t = p_c.tile([32, 128], F32, tag="ones")
        nc.vector.memset(ones_t[:], 0.0)
        nc.vector.memset(ones_t[0:1, :], 1.0)
        # ACT: absorb a DVE tick, then the activation-const DMA tick
        aw1 = p_c.tile([128, 2], F32, tag="aw1")
        nc.vector.memset(aw1[:], 0.0)
        aw2 = p_c.tile([128, 2], F32, tag="aw2")
        nc.scalar.copy(out=aw2[:], in_=aw1[:])
        aw3 = p_c.tile([128, 2], F32, tag="aw3")
        nc.scalar.activation(out=aw3[:], in_=aw2[:], func=Exp)

        qT = [p_qk.tile([128, T], F16, tag=f"qT{jt}", name=f"qT{jt}")
              for jt in range(2)]
        kTt = [p_qk.tile([128, T], F16, tag=f"kT{jt}", name=f"kT{jt}")
               for jt in range(2)]
        yT = [p_y.tile([128, T], F16, tag=f"yT{ct}", name=f"yT{ct}")
              for ct in range(2)]
        bv_t = p_c.tile([128, HPC * 65], F32, tag="bv")
        v_sb = [None] * NQ
        y_sb = {}      # (j, ct) -> [128, 128] f16 normalized y (q, cs-half)

        E_chunks = {}  # (h, i) -> list of (c0, cw, tile)
        E_masks = {}   # (h, i) -> masked diagonal [128, 128]

        def qk_move(ps, jt, tt, which):
            b_tile = bq_t if which == "q" else bk_t
            dest = qT if which == "q" else kTt
            nc.vector.tensor_scalar(
                out=dest[jt][:, 512 * tt:512 * (tt + 1)], in0=ps[:],
                scalar1=b_tile[:, jt:jt + 1], scalar2=None, op0=ADD)

        def emit_QK_group(ps_qk, jt, tt, which, tag="qk"):
            w_all = wq_all if which == "q" else wk_all
            ps = ps_qk.tile([128, 512], F32, tag=tag, name="psqk")
            for k in range(KT):
                nc.tensor.matmul(
                    out=ps[:],
                    lhsT=w_all[:, CS * k + 128 * jt:CS * k + 128 * (jt + 1)],
                    rhs=xt[k][:, 512 * tt:512 * (tt + 1)],
                    start=(k == 0), stop=(k == KT - 1),
                    skip_group_check=True)
            qk_move(ps, jt, tt, which)

        def emit_QK_pair_kmajor(ps_qk, jt, tt):
            """q and k groups for (jt, tt) with their k-loops interleaved so
            both track the xt[k] DMA arrivals (startup path)."""
            psq = ps_qk.tile([128, 512], F32, tag="qk", name="psq")
            psk = ps_qk.tile([128, 512], F32, tag="qk", name="psk")
            for k in range(KT):
                for ps, w_all in ((psq, wq_all), (psk, wk_all)):
                    nc.tensor.matmul(
                        out=ps[:],
                        lhsT=w_all[:, CS * k + 128 * jt:
                                   CS * k + 128 * (jt + 1)],
                        rhs=xt[k][:, 512 * tt:512 * (tt + 1)],
                        start=(k == 0), stop=(k == KT - 1),
                        skip_group_check=True)
            qk_move(psq, jt, tt, "q")
            qk_move(psk, jt, tt, "k")

        def emit_V_unit(ps_v, tt, tag="v"):
            ps = ps_v.tile([128, CS], F32, tag=tag, name="psv")
            for k in range(KT):
                nc.tensor.matmul(
                    out=ps[:],
                    lhsT=xt[k][:, 128 * tt:128 * (tt + 1)],
                    rhs=wv_all[:, CS * k:CS * (k + 1)],
                    start=(k == 0), stop=(k == KT - 1),
                    skip_group_check=True)
            vt = p_v.tile([128, HPC * 65], F16, tag=f"vt{tt}",
                          name=f"vt{tt}")
            nc.vector.tensor_tensor(
                out=vt[:].rearrange("p (h d) -> p h d", d=65)[:, :, 0:64],
                in0=ps[:].rearrange("p (h d) -> p h d", d=64),
                in1=bv_t[:].rearrange("p (h d) -> p h d", d=65)[:, :, 0:64],
                op=ADD)
            nc.vector.memset(
                vt[:].rearrange("p (h d) -> p h d", d=65)[:, :, 64:65], 1.0)
            v_sb[tt] = vt

        def emit_ST(ps_st, h, i, c0, c1):
            """Scores+exp for head h, k-row-block i, score columns [c0, c1).
            c0 must be max(qlo, c0)-normalized by the caller."""
            jt, hb = h // 2, 64 * (h % 2)
            qlo = 128 * i
            c0 = max(qlo, c0)
            cw = c1 - c0
            if cw <= 0:
                return
            slab = ps_st.tile([128, cw], F32, tag="st", name="slab")
            n0 = c0
            while n0 < c1:
                nw = min(512, c1 - n0)
                nc.tensor.matmul(
                    out=slab[:, n0 - c0:n0 - c0 + nw],
                    lhsT=kTt[jt][hb:hb + 64, qlo:qlo + 128],
                    rhs=qT[jt][hb:hb + 64, n0:n0 + nw],
                    start=True, stop=True, skip_group_check=True)
                n0 += nw
            e = p_e.tile([128, cw], F16, tag="E", name="e")
            nc.scalar.activation(out=e[:], in_=slab[:], func=Exp)
            if c0 == qlo:  # diagonal chunk: masked copy into its own tile
                md = p_md.tile([128, 128], F16, tag="md", name="md")
                nc.vector.tensor_tensor(out=md[:], in0=e[:, 0:128],
                                        in1=mask_t[:], op=MULT)
                E_masks[(h, i)] = md
            E_chunks.setdefault((h, i), []).append((c0, cw, e))

        def emit_PV_j(ps_pv, h, j):
            """y_ps[q=128, 65] for head h, q-block j, accumulated over
            k-blocks i=0..j with the exp'd scores as the stationary operand.
            Column 64 (V's ones-column) accumulates the softmax denominator."""
            qlo = 128 * j
            pvt = ps_pv.tile([128, 512], F32, tag="pv", name="pv")
            for i in range(j + 1):
                if i == j:
                    lhs = E_masks[(h, i)][:]
                else:
                    c0, cw, e = next(ch for ch in E_chunks[(h, i)]
                                     if ch[0] <= qlo < ch[0] + ch[1])
                    lhs = e[:, qlo - c0:qlo - c0 + 128]
                nc.tensor.matmul(
                    out=pvt[:, 0:65], lhsT=lhs,
                    rhs=v_sb[i][:, 65 * h:65 * h + 65],
                    start=(i == 0), stop=(i == j), skip_group_check=True)
            # normalize: per-partition (q) reciprocal of the denominator col
            rcp = p_r.tile([128, 1], F32, tag="rcp", name="rcp")
            nc.vector.reciprocal_approx_fast(out=rcp[:], in_=pvt[:, 64:65])
            ct, half = h // 2, 64 * (h % 2)
            key = (j, ct)
            if key not in y_sb:
                y_sb[key] = p_ysb.tile([128, 128], F16, tag=f"ysb{j}_{ct}",
                                       name=f"ysb{j}_{ct}")
            nc.vector.tensor_scalar(
                out=y_sb[key][:, half:half + 64], in0=pvt[:, 0:64],
                scalar1=rcp[:], scalar2=None, op0=MULT)

        def emit_transpose(ps_tr, j, ct):
            """yT[ct][:, 128j:128j+128] = transpose of y_sb[(j, ct)]."""
            tp = ps_tr.tile([128, 128], F16, tag="pv", name="tr")
            nc.tensor.transpose(out=tp[:], in_=y_sb[(j, ct)][:],
                                identity=eye_t[:])
            nc.vector.tensor_copy(out=yT[ct][:, 128 * j:128 * (j + 1)],
                                  in_=tp[:])

        def emit_out(ps_pv, tt, direct=False):
            # two half-tiles from the pv pool keep the slab pool's rotation
            # free of out-projection traffic; the two PSUM->SBUF halves run
            # concurrently on ACT and DVE
            pos = [ps_pv.tile([128, 512], F32, tag="pv", name=f"po{jt}")
                   for jt in range(2)]
            for jt in range(2):
                for ct in range(2):
                    nc.tensor.matmul(
                        out=pos[jt][:],
                        lhsT=yT[ct][:, 128 * tt:128 * (tt + 1)],
                        rhs=wo_all[:, C * ct + 512 * jt:
                                   C * ct + 512 * (jt + 1)],
                        start=(ct == 0), stop=(ct == 1),
                        skip_group_check=True)
            stg = p_o.tile([128, 1024], F16, tag="o", name="stg")
            nc.scalar.copy(out=stg[:, 0:512], in_=pos[0][:])
            nc.vector.tensor_copy(out=stg[:, 512:1024], in_=pos[1][:])
            nc.sync.dma_start(out=d["out"][128 * tt:128 * (tt + 1), :],
                              in_=stg[:])

        es_st = ExitStack()
        es_v = ExitStack()
        es_pv = ExitStack()
        with es_st, es_pv:
            ps_st = es_st.enter_context(
                tc.tile_pool(name="ps_st", bufs=2, space="PSUM"))
            with es_v:
                ps_qk = es_v.enter_context(
                    tc.tile_pool(name="ps_qk", bufs=2, space="PSUM"))
                ps_v = es_v.enter_context(
                    tc.tile_pool(name="ps_v", bufs=2, space="PSUM"))
                # absorb the bvo DMA tick on PE, then broadcast V-bias row
                # across 128 partitions with a K=1 ones matmul
                bv_ps = ps_v.tile([128, HPC * 65], F32, tag="v",
                                  name="bvps")
                nc.tensor.matmul(out=bv_ps[:], lhsT=ones_t[:],
                                 rhs=bvo_t[:], start=True, stop=True,
                                 skip_group_check=True)
                nc.vector.tensor_copy(out=bv_t[:], in_=bv_ps[:])
                # Q/K for jt0 tt0/tt1 first (unblocks head-0 scores panel 0);
                # the first pair runs k-major so it tracks the xt DMA stream
                emit_QK_pair_kmajor(ps_qk, 0, 0)
                # head-0 scores interleaved with the remaining jt0 QK groups
                # and the V units (jt1 QK moves to phase 1 where ACT is the
                # pacer and PE has slack)
                jt1_pool = [None]
                fillers = []
                for tt in (2, 3):
                    fillers.append(lambda tt=tt: emit_QK_group(
                        ps_qk, 0, tt, "q"))
                    fillers.append(lambda tt=tt: emit_QK_group(
                        ps_qk, 0, tt, "k"))
                for tt in range(12):
                    fillers.append(lambda tt=tt: emit_V_unit(ps_v, tt))
                jt1 = []
                for tt in range(4):
                    jt1.append(lambda tt=tt: emit_QK_group(
                        jt1_pool[0], 1, tt, "q", tag="pv"))
                    jt1.append(lambda tt=tt: emit_QK_group(
                        jt1_pool[0], 1, tt, "k", tag="pv"))
                fi = iter(fillers)

                def fill(n):
                    for _ in range(n):
                        f = next(fi, None)
                        if f:
                            f()

                # starter panels for i<4 need only the tt0 moves, so ACT
                # begins exp'ing right after the first QK pair lands
                for i in range(4):
                    emit_ST(ps_st, 0, i, 0, 512)
                emit_QK_pair_kmajor(ps_qk, 0, 1)
                for i in range(4):
                    emit_ST(ps_st, 0, i, 512, 1024)
                    fill(1)
                for i in range(4, 8):
                    emit_ST(ps_st, 0, i, 0, 1024)
                    fill(2)
                for i in range(NQ):
                    emit_ST(ps_st, 0, i, 1024, 2048)
                    fill(1)
                fill(len(fillers))
            # ps_v closed; phase 1 (head 1 scores + head-0 PV + jt1 QK)
            ps_pv = es_pv.enter_context(
                tc.tile_pool(name="ps_pv", bufs=4, space="PSUM"))
            jt1_pool[0] = ps_pv
            fj = iter(jt1)
            for i in range(NQ):
                emit_ST(ps_st, 1, i, 0, 1024)
                emit_ST(ps_st, 1, i, 1024, 2048)
                if i >= 8:
                    # late phase-1 steps are exp-paced with ACT slack: pull
                    # head-2's panel-0 slabs (need only the jt1 tt0/tt1
                    # moves, long done) forward to densify the exp stream
                    emit_ST(ps_st, 2, i - 8, 0, 1024)
                if i < 4:
                    emit_V_unit(ps_pv, 12 + i, tag="pv")
                f = next(fj, None)
                if f:
                    f()
                if i < 8:
                    for j in (2 * i, 2 * i + 1):
                        emit_PV_j(ps_pv, 0, j)
            for f in fj:
                f()
            ps_tr = ps_pv
            # merged phase 2+3: head-2 and head-3 scores interleave per
            # k-block, so the ACT-bound exp stream of both heads overlaps the
            # PE-heavy PV/transpose/out pipeline instead of serializing it
            tr0 = 0  # next (j, ct=0) transpose to emit
            for i in range(NQ):
                if i >= 3:
                    # deps are 2+ steps old: placing it first gives PE ready
                    # work while the new slabs wait on exp slot recycling
                    emit_out(ps_pv, i - 3)
                if i >= 1:
                    j = i - 1
                    emit_PV_j(ps_pv, 2, j)
                    emit_PV_j(ps_pv, 3, j)
                if i >= 8:
                    emit_ST(ps_st, 2, i, 0, 1024)
                emit_ST(ps_st, 2, i, 1024, 2048)
                emit_ST(ps_st, 3, i, 0, 1024)
                emit_ST(ps_st, 3, i, 1024, 2048)
                if i < 8:
                    for j in (2 * i, 2 * i + 1):
                        emit_PV_j(ps_pv, 1, j)
                if i >= 2:
                    emit_transpose(ps_tr, i - 2, 1)
                # transpose (j, 0) once heads 0 and 1 are normed
                while tr0 < NQ and tr0 <= 2 * i - 1:
                    emit_transpose(ps_tr, tr0, 0)
                    tr0 += 1
            emit_PV_j(ps_pv, 2, 15)
            emit_PV_j(ps_pv, 3, 15)
            for j in (14, 15):
                emit_transpose(ps_tr, j, 1)
            for tt in (13, 14, 15):
                emit_out(ps_pv, tt)


def _core_inputs(x, Wq, bq, Wk, bk, Wv, bv, Wo, core):
    b, g = core // HPC, core % HPC
    hs = slice(CS * g, CS * (g + 1))
    scale = np.float32(1.0 / np.sqrt(D))
    bvo = np.zeros((2, HPC * 65), np.float32)
    bvo[1, :] = 1.0
    for h in range(HPC):
        bvo[0, 65 * h:65 * h + 64] = bv[hs][64 * h:64 * h + 64]
    kk = np.arange(128)[:, None]
    qq = np.arange(128)[None, :]
    mask01 = (kk <= qq).astype(np.float16)
    return {
        "xT": np.ascontiguousarray(x[b].T).astype(np.float16),
        "wqT": np.ascontiguousarray((Wq[hs] * scale).T).astype(np.float16),
        "wkT": np.ascontiguousarray(Wk[hs].T).astype(np.float16),
        "wvT": np.ascontiguousarray(Wv[hs].T).astype(np.float16),
        "woT": np.ascontiguousarray(Wo[:, hs].T).astype(np.float16),
        "bq2": np.ascontiguousarray((bq[hs] * scale).reshape(2, 128).T,
                                    np.float32),
        "bk2": np.ascontiguousarray(bk[hs].reshape(2, 128).T, np.float32),
        "bvo": bvo,
        "mask": mask01,
        "eye": np.eye(128, dtype=np.float16),
    }


def get_program():
    global _PROGRAM
    if _PROGRAM is None:
        _PROGRAM = _build_program()
    return _PROGRAM


def make_in_maps(x, Wq, bq, Wk, bk, Wv, bv, Wo):
    return [_core_inputs(x, Wq, bq, Wk, bk, Wv, bv, Wo, core)
            for core in range(NCORES)]


def assemble(results, bo):
    out = np.zeros((B, T, C), np.float32)
    for core in range(NCORES):
        out[core // HPC] += results[core]["out"].astype(np.float32)
    out += bo[None, None, :]
    return out


def kernel(x, Wq, bq, Wk, bk, Wv, bv, Wo, bo):
    x = np.asarray(x, np.float32)
    Wq, bq = np.asarray(Wq, np.float32), np.asarray(bq, np.float32)
    Wk, bk = np.asarray(Wk, np.float32), np.asarray(bk, np.float32)
    Wv, bv = np.asarray(Wv, np.float32), np.asarray(bv, np.float32)
    Wo, bo = np.asarray(Wo, np.float32), np.asarray(bo, np.float32)
    nc = get_program()
    in_maps = make_in_maps(x, Wq, bq, Wk, bk, Wv, bv, Wo)
    res = run_bass_kernel_spmd(nc, in_maps, list(range(NCORES)))
    return assemble(res.results, bo)
